# revision 30
# baseline (speedup 1.0000x reference)
"""Batched quantize->matmul->dequantize kernel for 8 Trainium2 NeuronCores.

Problem: input0 [16,1024,1024] f32, input1 [16,1024,1024] f32.
  qa = clip(round(input0*10), -128, 127); qb likewise
  out = (qa @ qb) / 10            # batched, f32

Strategy: shard the batch dim across 8 cores (2 batches/core); each core runs
an identical Bass/Tile kernel with no communication.

Quantization: one multiply-by-10 with int8 output — the hardware f32->int8
conversion is round-to-nearest-even with saturation, which is exactly
jnp.clip(jnp.round(x*10), -128, 127) (verified on device incl. the
double-rounding and saturation edge cases). The int8 is then cast to bf16
for the PE: ints <= 128 are exact in bf16, products are exact in the PE's
multiply, and the fp32 PSUM accumulation of integer partial sums < 2^24 is
exact, so the matmul result matches the reference bit-for-bit (up to the
final x0.1 vs /10, <= 1 ulp).

Dequant (x0.1) is fused into the mandatory PSUM->SBUF eviction on the
scalar engine.

The A operand is laid out [b, K, M] host-side during sharding (the PE's
native stationary-operand layout: matmul computes lhsT.T @ rhs with the
contraction dim on partitions for both operands).

Schedule (all measured on HW): the wall is the serial 24 MiB DMA stream
(~400 GB/s) plus the PE-serial 256 matmuls (216 ns each, warm). Input DMAs
are issued before all output DMAs so the ring FIFOs give the ingest strict
priority; evicted outputs park in SBUF meanwhile. A warmup matmul chain and
mid-sweep LDWEIGHTS bursts keep the PE's HAM activity monitor from clock-
throttling (1.2 vs 2.4 GHz) across ingest-paced stalls. GPSIMD is left idle
on purpose: its tensor ops run ~15us/tile and its SBUF port lock stalls
concurrent DVE ops.
"""

import sys

if "/opt/trn_rl_repo" not in sys.path:
    sys.path.insert(0, "/opt/trn_rl_repo")

import numpy as np

import concourse.bass as bass
import concourse.mybir as mybir
import concourse.tile as tile
from concourse import bacc
from concourse.bass_utils import run_bass_kernel_spmd
from concourse.tile_rust import add_dep_helper

N_CORES = 8
B, M, K, N = 16, 1024, 1024, 1024
BPC = B // N_CORES  # batches per core
P = 128
KT = K // P  # k tiles per batch
MT = M // P  # m tiles per batch

DSCALE = 10.0
WSCALE = 10.0
OSCALE = 10.0

f32 = mybir.dt.float32
bf16 = mybir.dt.bfloat16
i8 = mybir.dt.int8


def _build_kernel(nc: bass.Bass):
    # A arrives pre-arranged [BPC, K, M]; B natural [BPC, K, N].
    a_dram = nc.dram_tensor("input0_t", [BPC, K, M], f32, kind="ExternalInput").ap()
    b_dram = nc.dram_tensor("input1", [BPC, K, N], f32, kind="ExternalInput").ap()
    c_dram = nc.dram_tensor("output", [BPC, M, N], f32, kind="ExternalOutput").ap()

    KP = KT // 2  # k-tile pairs: quant ops process two k-tiles at once

    with tile.TileContext(nc) as tc:
        with (
            tc.tile_pool(name="warm", bufs=1) as warm_pool,
            tc.tile_pool(name="a_f32", bufs=5) as a_pool,
            tc.tile_pool(name="b_f32", bufs=5) as b_pool,
            tc.tile_pool(name="a_i8", bufs=3) as ai_pool,
            tc.tile_pool(name="b_i8", bufs=3) as bi_pool,
            tc.tile_pool(name="qa", bufs=BPC * KP) as qa_pool,
            tc.tile_pool(name="qb", bufs=BPC * KP) as qb_pool,
            tc.tile_pool(name="psum", bufs=4, space="PSUM") as psum_pool,
            tc.tile_pool(name="c_f32", bufs=3) as c_pool,
        ):
            # PE warmup: keep the PE busy from t~0 so the HAM clock gate is
            # released (2.4 GHz) by the time real matmuls are ready, instead
            # of paying ~2x cadence on the first ~3.4us of real work.
            wsrc = warm_pool.tile([P, 512], bf16)
            nc.gpsimd.memset(wsrc[:], 0.0)
            wps = psum_pool.tile([P, 512], f32, tag="ps", name="wps")
            for _ in range(46):
                nc.tensor.matmul(wps[:], wsrc[:, :P], wsrc[:], start=True, stop=True)

            # Emit ALL input loads + quant first: the Sync queue then
            # issues every input DMA before any output DMA, and the DMA ring
            # FIFOs give the input stream strict priority -- the critical
            # path is the serial 16 MiB input ingest, so outputs must not
            # steal bandwidth from it. Evicted outputs park in SBUF (ct
            # tiles) until the input stream drains.
            qa = [[] for _ in range(BPC)]
            qb = [[] for _ in range(BPC)]
            last_in_dma = None
            for b in range(BPC):
                for kp in range(KP):
                    at = a_pool.tile([P, 2 * M], f32, tag="at", name=f"at{b}_{kp}")
                    for t in range(2):
                        last_in_dma = nc.sync.dma_start(
                            out=at[:, t * M : (t + 1) * M],
                            in_=a_dram[b, (2 * kp + t) * P : (2 * kp + t + 1) * P, :],
                        )
                    ai = ai_pool.tile([P, 2 * M], i8, tag="ai", name=f"ai{b}_{kp}")
                    qat = qa_pool.tile([P, 2 * M], bf16, tag="qa", name=f"qa{b}_{kp}")
                    # f32->int8 convert = RNE + saturate == clip(round(10x))
                    if kp == KP - 1:
                        # the last pair gates the batch's whole PSUM tail:
                        # quantize per 512 KiB half so each DVE op fires the
                        # moment its half of the DMA lands
                        for t in range(2):
                            sl = slice(t * M, (t + 1) * M)
                            nc.vector.tensor_scalar_mul(ai[:, sl], at[:, sl], DSCALE)
                            nc.vector.tensor_copy(out=qat[:, sl], in_=ai[:, sl])
                    else:
                        nc.vector.tensor_scalar_mul(ai[:], at[:], DSCALE)
                        nc.vector.tensor_copy(out=qat[:], in_=ai[:])
                    qa[b].append(qat)

                    bt = b_pool.tile([P, 2 * N], f32, tag="bt", name=f"bt{b}_{kp}")
                    for t in range(2):
                        last_in_dma = nc.sync.dma_start(
                            out=bt[:, t * N : (t + 1) * N],
                            in_=b_dram[b, (2 * kp + t) * P : (2 * kp + t + 1) * P, :],
                        )
                    bi = bi_pool.tile([P, 2 * N], i8, tag="bi", name=f"bi{b}_{kp}")
                    qbt = qb_pool.tile([P, 2 * N], bf16, tag="qb", name=f"qb{b}_{kp}")
                    if kp == KP - 1:
                        # same half-granularity, all on DVE (ACT is slower
                        # and busy; DVE tracks the stream with zero slack)
                        for t in range(2):
                            sl = slice(t * N, (t + 1) * N)
                            nc.vector.tensor_scalar_mul(bi[:, sl], bt[:, sl], WSCALE)
                            nc.vector.tensor_copy(out=qbt[:, sl], in_=bi[:, sl])
                    else:
                        nc.vector.tensor_scalar_mul(bi[:], bt[:], WSCALE)
                        if b == 0 and kp == 0:
                            # first pair: cast on DVE so the first real
                            # matmul starts early
                            nc.vector.tensor_copy(out=qbt[:], in_=bi[:])
                        else:
                            nc.scalar.copy(qbt[:], bi[:])
                    qb[b].append(qbt)

            for b in range(BPC):
                # k-outer over groups of m-tiles: PE consumes each k pair as
                # it streams in instead of needing the whole batch resident
                # before finishing any PSUM accumulation. Batch 0 uses two
                # 4-tile groups (minimal post-ingest PE tail); the last batch
                # ends with a 1-tile group so a single eviction gates the
                # final output DMA.
                groups = ((0, 4), (4, 4)) if b < BPC - 1 else ((0, 3), (3, 4), (7, 1))
                for m0, gsz in groups:
                    ps = [
                        psum_pool.tile([P, N], f32, tag="ps", name=f"ps_{b}_{m0}_{i}")
                        for i in range(gsz)
                    ]
                    for k in range(KT):
                        kp, t = divmod(k, 2)
                        for mi in range(gsz):
                            m = m0 + mi
                            lhsT = qa[b][kp][:, t * M + m * P : t * M + (m + 1) * P]
                            for nh in range(2):
                                nc.tensor.matmul(
                                    ps[mi][:, nh * 512 : (nh + 1) * 512],
                                    lhsT,
                                    qb[b][kp][
                                        :, t * N + nh * 512 : t * N + (nh + 1) * 512
                                    ],
                                    start=(k == 0),
                                    stop=(k == KT - 1),
                                )
                        if b == 0 and m0 == 0 and k in (1, 3, 5):
                            # the first sweep is paced by the input stream;
                            # these weight-load bursts keep the PE activity
                            # monitor from re-throttling the clock while the
                            # PE waits for the next k pair (no PSUM writes)
                            for _ in range(12):
                                nc.tensor.ldweights(wsrc[:, :P])
                    ct = c_pool.tile([P, gsz * N], f32, tag="ct", name=f"ct_{b}_{m0}")
                    ct3 = ct[:].rearrange("p (g n) -> p g n", g=gsz)
                    final = b == BPC - 1 and (m0, gsz) == groups[-1]
                    for h in range(gsz):
                        m = m0 + h
                        # dequant fused into the PSUM->SBUF eviction; the
                        # very last tile evicts in halves so its output DMA
                        # starts half an eviction earlier
                        nhalves = 2 if final else 1
                        for q in range(nhalves):
                            sl = slice(q * N // nhalves, (q + 1) * N // nhalves)
                            nc.scalar.activation(
                                ct3[:, h, sl],
                                ps[h][:, sl],
                                mybir.ActivationFunctionType.Copy,
                                scale=1.0 / OSCALE,
                            )
                            od = nc.sync.dma_start(
                                out=c_dram[b, m * P : (m + 1) * P, sl],
                                in_=ct3[:, h, sl],
                            )
                            # outputs issue only after the whole input stream
                            # has been issued: ring FIFOs then transfer every
                            # input byte before the first output byte.
                            add_dep_helper(
                                od.ins,
                                last_in_dma.ins,
                                sync=False,
                                reason="outputs after input stream",
                            )


_NC_CACHE = None


def _get_nc():
    global _NC_CACHE
    if _NC_CACHE is None:
        nc = bacc.Bacc("TRN2", target_bir_lowering=False, debug=False,
                       num_devices=N_CORES)
        _build_kernel(nc)
        nc.compile()
        _NC_CACHE = nc
    return _NC_CACHE


def _make_in_maps(input0: np.ndarray, input1: np.ndarray):
    in_maps = []
    for c in range(N_CORES):
        sl = slice(c * BPC, (c + 1) * BPC)
        a_t = np.ascontiguousarray(input0[sl].transpose(0, 2, 1))
        in_maps.append(
            {"input0_t": a_t, "input1": np.ascontiguousarray(input1[sl])}
        )
    return in_maps


def kernel(input0, input1, **run_kwargs):
    input0 = np.asarray(input0, dtype=np.float32)
    input1 = np.asarray(input1, dtype=np.float32)
    assert input0.shape == (B, M, K) and input1.shape == (B, K, N)

    nc = _get_nc()
    in_maps = _make_in_maps(input0, input1)
    res = None
    for attempt in range(3):
        try:
            res = run_bass_kernel_spmd(
                nc, in_maps, core_ids=list(range(N_CORES)), **run_kwargs,
            )
            break
        except Exception:
            if attempt == 2:
                raise
    assert res is not None
    out = np.concatenate(
        [res.results[c]["output"] for c in range(N_CORES)], axis=0
    )
    if run_kwargs:
        return out, res
    return out


if __name__ == "__main__":
    a = np.random.randn(B, M, K).astype(np.float32)
    bm = np.random.randn(B, K, N).astype(np.float32)
    out = kernel(a, bm)
    print("out", out.shape, out.dtype)


# revision 31
# speedup vs baseline: 1.0341x; 1.0341x over previous
"""Batched quantize->matmul->dequantize kernel for 8 Trainium2 NeuronCores.

Problem: input0 [16,1024,1024] f32, input1 [16,1024,1024] f32.
  qa = clip(round(input0*10), -128, 127); qb likewise
  out = (qa @ qb) / 10            # batched, f32

Strategy: shard the batch dim across 8 cores (2 batches/core); each core runs
an identical Bass/Tile kernel with no communication.

Quantization: one multiply-by-10 with int8 output — the hardware f32->int8
conversion is round-to-nearest-even with saturation, which is exactly
jnp.clip(jnp.round(x*10), -128, 127) (verified on device incl. the
double-rounding and saturation edge cases). The int8 is then cast to bf16
for the PE: ints <= 128 are exact in bf16, products are exact in the PE's
multiply, and the fp32 PSUM accumulation of integer partial sums < 2^24 is
exact, so the matmul result matches the reference bit-for-bit (up to the
final x0.1 vs /10, <= 1 ulp).

Dequant (x0.1) is fused into the mandatory PSUM->SBUF eviction on the
scalar engine.

The A operand is laid out [b, K, M] host-side during sharding (the PE's
native stationary-operand layout: matmul computes lhsT.T @ rhs with the
contraction dim on partitions for both operands).

Schedule (all measured on HW): the wall is the serial 24 MiB DMA stream
(~400 GB/s) plus the PE-serial 256 matmuls (216 ns each, warm). Input DMAs
are issued before all output DMAs so the ring FIFOs give the ingest strict
priority; evicted outputs park in SBUF meanwhile. A warmup matmul chain and
mid-sweep LDWEIGHTS bursts keep the PE's HAM activity monitor from clock-
throttling (1.2 vs 2.4 GHz) across ingest-paced stalls. GPSIMD is left idle
on purpose: its tensor ops run ~15us/tile and its SBUF port lock stalls
concurrent DVE ops.
"""

import sys

if "/opt/trn_rl_repo" not in sys.path:
    sys.path.insert(0, "/opt/trn_rl_repo")

import numpy as np

import concourse.bass as bass
import concourse.mybir as mybir
import concourse.tile as tile
from concourse import bacc
from concourse.bass_utils import run_bass_kernel_spmd
from concourse.tile_rust import add_dep_helper

N_CORES = 8
B, M, K, N = 16, 1024, 1024, 1024
BPC = B // N_CORES  # batches per core
P = 128
KT = K // P  # k tiles per batch
MT = M // P  # m tiles per batch

DSCALE = 10.0
WSCALE = 10.0
OSCALE = 10.0

f32 = mybir.dt.float32
bf16 = mybir.dt.bfloat16
i8 = mybir.dt.int8


def _build_kernel(nc: bass.Bass):
    # A arrives pre-arranged [BPC, K, M]; B natural [BPC, K, N].
    a_dram = nc.dram_tensor("input0_t", [BPC, K, M], f32, kind="ExternalInput").ap()
    b_dram = nc.dram_tensor("input1", [BPC, K, N], f32, kind="ExternalInput").ap()
    c_dram = nc.dram_tensor("output", [BPC, M, N], f32, kind="ExternalOutput").ap()

    KP = KT // 2  # k-tile pairs: quant ops process two k-tiles at once

    with tile.TileContext(nc) as tc:
        with (
            tc.tile_pool(name="warm", bufs=1) as warm_pool,
            tc.tile_pool(name="a_f32", bufs=5) as a_pool,
            tc.tile_pool(name="b_f32", bufs=5) as b_pool,
            tc.tile_pool(name="a_i8", bufs=3) as ai_pool,
            tc.tile_pool(name="b_i8", bufs=3) as bi_pool,
            tc.tile_pool(name="qa", bufs=BPC * KP) as qa_pool,
            tc.tile_pool(name="qb", bufs=BPC * KP) as qb_pool,
            tc.tile_pool(name="psum", bufs=4, space="PSUM") as psum_pool,
            tc.tile_pool(name="c_f32", bufs=3) as c_pool,
        ):
            # PE warmup: keep the PE busy from t~0 so the HAM clock gate is
            # released (2.4 GHz) by the time real matmuls are ready, instead
            # of paying ~2x cadence on the first ~3.4us of real work.
            wsrc = warm_pool.tile([P, 512], bf16)
            nc.gpsimd.memset(wsrc[:], 0.0)
            wps = psum_pool.tile([P, 512], f32, tag="ps", name="wps")
            for _ in range(46):
                nc.tensor.matmul(wps[:], wsrc[:, :P], wsrc[:], start=True, stop=True)

            # Emit ALL input loads + quant first: the Sync queue then
            # issues every input DMA before any output DMA, and the DMA ring
            # FIFOs give the input stream strict priority -- the critical
            # path is the serial 16 MiB input ingest, so outputs must not
            # steal bandwidth from it. Evicted outputs park in SBUF (ct
            # tiles) until the input stream drains.
            qa = [[] for _ in range(BPC)]
            qb = [[] for _ in range(BPC)]
            last_in_dma = None
            for b in range(BPC):
                for kp in range(KP):
                    at = a_pool.tile([P, 2 * M], f32, tag="at", name=f"at{b}_{kp}")
                    for t in range(2):
                        last_in_dma = nc.sync.dma_start(
                            out=at[:, t * M : (t + 1) * M],
                            in_=a_dram[b, (2 * kp + t) * P : (2 * kp + t + 1) * P, :],
                        )
                    ai = ai_pool.tile([P, 2 * M], i8, tag="ai", name=f"ai{b}_{kp}")
                    qat = qa_pool.tile([P, 2 * M], bf16, tag="qa", name=f"qa{b}_{kp}")
                    # f32->int8 convert = RNE + saturate == clip(round(10x))
                    if kp == KP - 1:
                        # the last pair gates the batch's whole PSUM tail:
                        # quantize per 512 KiB half so each DVE op fires the
                        # moment its half of the DMA lands
                        for t in range(2):
                            sl = slice(t * M, (t + 1) * M)
                            nc.vector.tensor_scalar_mul(ai[:, sl], at[:, sl], DSCALE)
                            nc.vector.tensor_copy(out=qat[:, sl], in_=ai[:, sl])
                    else:
                        nc.vector.tensor_scalar_mul(ai[:], at[:], DSCALE)
                        nc.vector.tensor_copy(out=qat[:], in_=ai[:])
                    qa[b].append(qat)

                    bt = b_pool.tile([P, 2 * N], f32, tag="bt", name=f"bt{b}_{kp}")
                    for t in range(2):
                        last_in_dma = nc.sync.dma_start(
                            out=bt[:, t * N : (t + 1) * N],
                            in_=b_dram[b, (2 * kp + t) * P : (2 * kp + t + 1) * P, :],
                        )
                    bi = bi_pool.tile([P, 2 * N], i8, tag="bi", name=f"bi{b}_{kp}")
                    qbt = qb_pool.tile([P, 2 * N], bf16, tag="qb", name=f"qb{b}_{kp}")
                    if kp == KP - 1:
                        # same half-granularity, all on DVE (ACT is slower
                        # and busy; DVE tracks the stream with zero slack)
                        for t in range(2):
                            sl = slice(t * N, (t + 1) * N)
                            nc.vector.tensor_scalar_mul(bi[:, sl], bt[:, sl], WSCALE)
                            nc.vector.tensor_copy(out=qbt[:, sl], in_=bi[:, sl])
                    else:
                        nc.vector.tensor_scalar_mul(bi[:], bt[:], WSCALE)
                        if b == 0 and kp == 0:
                            # first pair: cast on DVE so the first real
                            # matmul starts early
                            nc.vector.tensor_copy(out=qbt[:], in_=bi[:])
                        else:
                            nc.scalar.copy(qbt[:], bi[:])
                    qb[b].append(qbt)

            for b in range(BPC):
                # k-outer over groups of m-tiles: PE consumes each k pair as
                # it streams in instead of needing the whole batch resident
                # before finishing any PSUM accumulation. Batch 0 uses two
                # 4-tile groups (minimal post-ingest PE tail); the last batch
                # ends with a 1-tile group so a single eviction gates the
                # final output DMA.
                groups = ((0, 4), (4, 3), (7, 1)) if b < BPC - 1 else ((0, 3), (3, 4), (7, 1))
                for m0, gsz in groups:
                    ps = [
                        psum_pool.tile([P, N], f32, tag="ps", name=f"ps_{b}_{m0}_{i}")
                        for i in range(gsz)
                    ]
                    for k in range(KT):
                        kp, t = divmod(k, 2)
                        for mi in range(gsz):
                            m = m0 + mi
                            lhsT = qa[b][kp][:, t * M + m * P : t * M + (m + 1) * P]
                            for nh in range(2):
                                nc.tensor.matmul(
                                    ps[mi][:, nh * 512 : (nh + 1) * 512],
                                    lhsT,
                                    qb[b][kp][
                                        :, t * N + nh * 512 : t * N + (nh + 1) * 512
                                    ],
                                    start=(k == 0),
                                    stop=(k == KT - 1),
                                )
                        if b == 0 and m0 == 0 and k in (1, 3, 5):
                            # the first sweep is paced by the input stream;
                            # these weight-load bursts keep the PE activity
                            # monitor from re-throttling the clock while the
                            # PE waits for the next k pair (no PSUM writes)
                            for _ in range(12):
                                nc.tensor.ldweights(wsrc[:, :P])
                    ct = c_pool.tile([P, gsz * N], f32, tag="ct", name=f"ct_{b}_{m0}")
                    ct3 = ct[:].rearrange("p (g n) -> p g n", g=gsz)
                    final = b == BPC - 1 and (m0, gsz) == groups[-1]
                    for h in range(gsz):
                        m = m0 + h
                        # dequant fused into the PSUM->SBUF eviction; the
                        # very last tile evicts in halves so its output DMA
                        # starts half an eviction earlier
                        nhalves = 2 if final else 1
                        for q in range(nhalves):
                            sl = slice(q * N // nhalves, (q + 1) * N // nhalves)
                            nc.scalar.activation(
                                ct3[:, h, sl],
                                ps[h][:, sl],
                                mybir.ActivationFunctionType.Copy,
                                scale=1.0 / OSCALE,
                            )
                            od = nc.sync.dma_start(
                                out=c_dram[b, m * P : (m + 1) * P, sl],
                                in_=ct3[:, h, sl],
                            )
                            # outputs issue only after the whole input stream
                            # has been issued: ring FIFOs then transfer every
                            # input byte before the first output byte.
                            add_dep_helper(
                                od.ins,
                                last_in_dma.ins,
                                sync=False,
                                reason="outputs after input stream",
                            )


_NC_CACHE = None


def _get_nc():
    global _NC_CACHE
    if _NC_CACHE is None:
        nc = bacc.Bacc("TRN2", target_bir_lowering=False, debug=False,
                       num_devices=N_CORES)
        _build_kernel(nc)
        nc.compile()
        _NC_CACHE = nc
    return _NC_CACHE


def _make_in_maps(input0: np.ndarray, input1: np.ndarray):
    in_maps = []
    for c in range(N_CORES):
        sl = slice(c * BPC, (c + 1) * BPC)
        a_t = np.ascontiguousarray(input0[sl].transpose(0, 2, 1))
        in_maps.append(
            {"input0_t": a_t, "input1": np.ascontiguousarray(input1[sl])}
        )
    return in_maps


def kernel(input0, input1, **run_kwargs):
    input0 = np.asarray(input0, dtype=np.float32)
    input1 = np.asarray(input1, dtype=np.float32)
    assert input0.shape == (B, M, K) and input1.shape == (B, K, N)

    nc = _get_nc()
    in_maps = _make_in_maps(input0, input1)
    res = None
    for attempt in range(3):
        try:
            res = run_bass_kernel_spmd(
                nc, in_maps, core_ids=list(range(N_CORES)), **run_kwargs,
            )
            break
        except Exception:
            if attempt == 2:
                raise
    assert res is not None
    out = np.concatenate(
        [res.results[c]["output"] for c in range(N_CORES)], axis=0
    )
    if run_kwargs:
        return out, res
    return out


if __name__ == "__main__":
    a = np.random.randn(B, M, K).astype(np.float32)
    bm = np.random.randn(B, K, N).astype(np.float32)
    out = kernel(a, bm)
    print("out", out.shape, out.dtype)
